# revision 18
# baseline (speedup 1.0000x reference)
"""Paged attention (decode) on 8 Trainium2 NeuronCores — block-granularity
gathers.

Sharding: tensor-parallel over KV heads — core h owns kv head h, its 4
query heads, and the per-head slices of both caches.

vs the 4-token-row version: gathers move whole 16-token blocks (4 KB
descriptors instead of 1 KB, 4.4x fewer descriptors) and only the blocks
each sequence actually references (no padding to 512-token groups).

Layouts:
  K blocks -> krows [NB, 2048] bf16, row b = K[b] as [tok, d] flattened.
     transpose-gathered (bin-packed across seqs, <=768 idxs per gather)
     into KT tiles [d=128, j(16), pos]: KT[d, j, i] = K[blk_i][j*128+d].
  V blocks -> vrows [NB, 2048] bf16, same row layout.
     plain-gathered per seq into V tiles [pos%128, pos//128, 2048].
  chunk = up to 128 consecutive blocks of one seq (<=2048 tokens).
  QK per (chunk, j):  psS[0:mc, c*16+j, q] = KT_slab[d, mc].T @ Q[d, 4]
  exp per chunk (partial partitions: stale PSUM is never read)
  PV per (chunk, j):  psO[:, s*4+q] += V_slab[0:mc, d] . P[0:mc, 4]
  denominators: ones.T @ P in two matmuls (full chunks + partial last)
"""

import numpy as np
import ml_dtypes

import concourse.bass as bass
import concourse.mybir as mybir
import concourse.tile as tile
from concourse import bacc
from concourse.bass_utils import run_bass_kernel_spmd

BF16 = ml_dtypes.bfloat16
BLOCK_SIZE = 16
BIN_CAP = 768               # blocks per K gather (HW gather limit ~832)


def _plan(ctx, B, bin_cap=None):
    """Bin-pack seqs' block lists into K gathers; lay out idx streams.

    Returns dict with per-seq (nb, C, m, bin id, a=offset in bin) and
    per-bin (klen padded to 128); plus V idx slot offsets (16-aligned).
    """
    cap = bin_cap or BIN_CAP
    nb = [-(-int(c) // BLOCK_SIZE) for c in ctx]
    bins = []                # list of [seq ids]
    fill = []                # current block count per bin
    binof = [0] * B
    aoff = [0] * B
    for s in range(B):
        placed = False
        for g in range(len(bins)):
            if fill[g] + nb[s] <= cap:
                binof[s], aoff[s] = g, fill[g]
                bins[g].append(s)
                fill[g] += nb[s]
                placed = True
                break
        if not placed:
            binof[s], aoff[s] = len(bins), 0
            bins.append([s])
            fill.append(nb[s])
    klen = [-(-f // 128) * 128 for f in fill]
    kboff = np.cumsum([0] + [k // 16 for k in klen])[:-1]  # idx slot offsets
    voff = np.cumsum([0] + [-(-n // 16) for n in nb])[:-1]
    return {
        "nb": nb, "bins": bins, "klen": klen,
        "kboff": [int(x) for x in kboff], "aoff": aoff, "binof": binof,
        "voff": [int(x) for x in voff],
        "nkslot": int(sum(k // 16 for k in klen)),
        "nvslot": int(sum(-(-n // 16) for n in nb)),
    }


def _build_program(n_blocks, n_seqs, plan, repeat=1, single_packet=True,
                   kq=0, vq=0, kbufs=4, vbufs=8, mode="full"):
    D = 128
    f32 = mybir.dt.float32
    bf16 = mybir.dt.bfloat16
    do_k = mode in ("full", "gather", "gatherK")
    do_v = mode in ("full", "gather", "gatherV")
    do_compute = mode == "full"
    nb, bins, klen = plan["nb"], plan["bins"], plan["klen"]
    kboff, aoff, voff = plan["kboff"], plan["aoff"], plan["voff"]
    nkslot, nvslot = plan["nkslot"], plan["nvslot"]

    nc = bacc.Bacc("TRN2", target_bir_lowering=False, debug=False)
    k_d = nc.dram_tensor("krows", [n_blocks, 2048], bf16, kind="ExternalInput")
    v_d = nc.dram_tensor("vrows", [n_blocks, 2048], bf16, kind="ExternalInput")
    q_d = nc.dram_tensor("qt", [D, n_seqs, 4], bf16, kind="ExternalInput")
    ik_d = nc.dram_tensor("idxk", [128, nkslot], mybir.dt.int16,
                          kind="ExternalInput")
    iv_d = nc.dram_tensor("idxv", [128, nvslot], mybir.dt.int16,
                          kind="ExternalInput")
    m_d = nc.dram_tensor("masks", [128, n_seqs, 16, 4], bf16,
                         kind="ExternalInput")
    o_d = nc.dram_tensor("out", [D, n_seqs * 4], f32, kind="ExternalOutput")

    with tile.TileContext(nc) as tc:
        with (
            tc.tile_pool(name="const", bufs=1) as const,
            tc.tile_pool(name="kp", bufs=kbufs) as kp,
            tc.tile_pool(name="vp", bufs=vbufs) as vp,
            tc.tile_pool(name="pp", bufs=2) as pp,
            tc.tile_pool(name="psS", bufs=2, space="PSUM") as psSp,
            tc.tile_pool(name="psO", bufs=1, space="PSUM") as psOp,
            tc.tile_pool(name="psD", bufs=2, space="PSUM") as psDp,
            tc.tile_pool(name="psR", bufs=1, space="PSUM") as psRp,
        ):
            qt = const.tile([D, n_seqs, 4], bf16)
            idxk = const.tile([128, nkslot], mybir.dt.int16)
            idxv = const.tile([128, nvslot], mybir.dt.int16)
            masks = const.tile([128, n_seqs, 16, 4], bf16)
            ones = const.tile([128, 1], bf16)
            onesr = const.tile([1, 128], f32)
            dsb = const.tile([1, n_seqs, 4], f32)
            outsb = const.tile([D, n_seqs * 4], f32)
            recs = const.tile([D, n_seqs * 4], f32)
            nc.sync.dma_start(qt[:], q_d[:])
            nc.sync.dma_start(idxk[:], ik_d[:])
            nc.sync.dma_start(idxv[:], iv_d[:])
            nc.sync.dma_start(masks[:], m_d[:])
            nc.vector.memset(ones[:], 1.0)
            nc.vector.memset(onesr[:], 1.0)
            if not do_compute:
                nc.vector.memset(dsb[:], 1.0)
                nc.vector.memset(outsb[:], 0.0)

            psO = psOp.tile([D, n_seqs * 4], f32)
            scale = float(1.0 / np.sqrt(np.float32(D)).astype(np.float32))

            def emit_seq(s, k_t):
                n = nb[s]
                C = -(-n // 128)
                m = n - (C - 1) * 128
                a = aoff[s]
                if do_v:
                    v_t = vp.tile([128, C, 2048], bf16, tag="v")
                    nc.gpsimd.dma_gather(
                        v_t[:], v_d[:],
                        idxv[:, voff[s]:voff[s] + (-(-n // 16))],
                        n, n, 2048,
                        single_packet=single_packet, queue_num=vq)
                if not do_compute:
                    return

                psS = psSp.tile([128, C * 16, 4], f32, tag="psS")
                rhs_q = qt[:, s, :]
                for c in range(C):
                    mc = 128 if c < C - 1 else m
                    base = a + c * 128
                    for j in range(16):
                        nc.tensor.matmul(
                            psS[0:mc, c * 16 + j, :],
                            k_t[:, j, base:base + mc],
                            rhs_q, start=True, stop=True)

                p_t = pp.tile([128, C * 16, 4], bf16, tag="p")
                for c in range(C):
                    mc = 128 if c < C - 1 else m
                    nc.scalar.activation(
                        p_t[0:mc, c * 16:(c + 1) * 16, :],
                        psS[0:mc, c * 16:(c + 1) * 16, :],
                        mybir.ActivationFunctionType.Exp,
                        scale=scale)
                nc.vector.tensor_mul(
                    p_t[0:m, (C - 1) * 16:C * 16, :],
                    p_t[0:m, (C - 1) * 16:C * 16, :],
                    masks[0:m, s, :, :])

                for c in range(C):
                    mc = 128 if c < C - 1 else m
                    for j in range(16):
                        nc.tensor.matmul(
                            psO[:, s * 4:(s + 1) * 4],
                            v_t[0:mc, c, j * 128:(j + 1) * 128],
                            p_t[0:mc, c * 16 + j, :],
                            start=(c == 0 and j == 0),
                            stop=(c == C - 1 and j == 15))

                psD = psDp.tile([1, 4, C * 16], f32, tag="psD")
                if C > 1:
                    nc.tensor.matmul(
                        psD[:, :, 0:(C - 1) * 16], ones[:],
                        p_t[:, 0:(C - 1) * 16, :].transpose([0, 2, 1]),
                        start=True, stop=True)
                nc.tensor.matmul(
                    psD[:, :, (C - 1) * 16:C * 16], ones[0:m, :],
                    p_t[0:m, (C - 1) * 16:C * 16, :].transpose([0, 2, 1]),
                    start=True, stop=True)
                nc.vector.tensor_reduce(dsb[:, s, :], psD[:],
                                        mybir.AxisListType.X,
                                        mybir.AluOpType.add)

            for _ in range(repeat):
                for g, bin_seqs in enumerate(bins):
                    k_t = None
                    if do_k:
                        L = klen[g]
                        k_t = kp.tile([D, 16, L], bf16, tag="k")
                        nc.gpsimd.dma_gather(
                            k_t[:], k_d[:],
                            idxk[:, kboff[g]:kboff[g] + L // 16],
                            L, L, 2048, transpose=True,
                            single_packet=single_packet, queue_num=kq)
                    for s in bin_seqs:
                        emit_seq(s, k_t)

            # epilogue: replicate denominators to all partitions, divide
            psR = psRp.tile([128, n_seqs * 4], f32)
            nc.tensor.matmul(psR[:], onesr[:], dsb[0:1, :, :],
                             start=True, stop=True)
            nc.vector.reciprocal(recs[:], psR[:])
            if do_compute:
                nc.vector.tensor_mul(outsb[:], psO[:], recs[:])
            nc.sync.dma_start(o_d[:], outsb[:])

    nc.compile()
    return nc


def prepare(query, key_cache, value_cache, block_tables, context_lens,
            repeat=1, bin_cap=None):
    query = np.asarray(query)
    key_cache = np.asarray(key_cache)
    value_cache = np.asarray(value_cache)
    block_tables = np.asarray(block_tables)
    context_lens = np.asarray(context_lens)

    nb_tot, kvh, dx, bs, x = key_cache.shape
    D = dx * x
    B, H, _ = query.shape
    qpk = H // kvh
    assert D == 128 and bs == BLOCK_SIZE and qpk == 4 and B * 4 <= 128

    ctx = context_lens.astype(np.int64)
    plan = _plan(ctx, B, bin_cap=bin_cap)
    nb = plan["nb"]

    # K idx stream (bin-packed), V idx stream (per-seq, 16-aligned)
    kvals = np.zeros(plan["nkslot"] * 16, dtype=np.int16)
    for g, bin_seqs in enumerate(plan["bins"]):
        base = plan["kboff"][g] * 16
        o = base
        for s in bin_seqs:
            kvals[o:o + nb[s]] = block_tables[s, :nb[s]].astype(np.int16)
            o += nb[s]
    idxk = np.ascontiguousarray(np.tile(np.ascontiguousarray(
        kvals.reshape(plan["nkslot"], 16).T), (8, 1)))

    vvals = np.zeros(plan["nvslot"] * 16, dtype=np.int16)
    for s in range(B):
        base = plan["voff"][s] * 16
        vvals[base:base + nb[s]] = block_tables[s, :nb[s]].astype(np.int16)
    idxv = np.ascontiguousarray(np.tile(np.ascontiguousarray(
        vvals.reshape(plan["nvslot"], 16).T), (8, 1)))

    # masks [128, B, 16, 4]: validity of the LAST chunk of each seq.
    # partition p = block (C-1)*128+p of the seq, col (j, q):
    # valid iff ((C-1)*128+p)*16 + j < ctx
    p_ar = np.arange(128)[:, None]
    j_ar = np.arange(16)[None, :]
    masks = np.zeros((128, B, 16, 4), dtype=np.float32)
    for s in range(B):
        C = -(-nb[s] // 128)
        tok = ((C - 1) * 128 + p_ar) * 16 + j_ar     # [128, 16]
        masks[:, s, :, 0] = (tok < int(ctx[s])).astype(np.float32)
    masks[:, :, :, 1:] = masks[:, :, :, 0:1]
    masks = masks.astype(BF16)

    in_maps = []
    for h in range(kvh):
        kc = key_cache[:, h]                          # [NB, dx, bs, x]
        K = np.ascontiguousarray(kc.transpose(0, 2, 1, 3)).reshape(nb_tot, -1)
        krows = K.astype(BF16)                        # [NB, 2048] tok-major

        vc = value_cache[:, h]                        # [NB, D, bs]
        V = np.ascontiguousarray(vc.transpose(0, 2, 1)).reshape(nb_tot, -1)
        vrows = V.astype(BF16)                        # [NB, 2048] tok-major

        qh = query[:, 4 * h:4 * h + 4, :]             # [B, 4, D]
        qt = np.ascontiguousarray(
            qh.transpose(2, 0, 1)).astype(BF16)       # [D, B, 4]

        in_maps.append({
            "krows": krows, "vrows": vrows, "qt": qt,
            "idxk": idxk, "idxv": idxv, "masks": masks,
        })

    build_args = (nb_tot, B, plan)
    globals()["_last_build_args"] = build_args
    nc = _build_program(*build_args, repeat=repeat)
    return nc, in_maps, (B, H, D, kvh)


def assemble(res, meta):
    B, H, D, kvh = meta
    out = np.empty((B, H, D), dtype=np.float32)
    for h in range(kvh):
        o = res[h]["out"]                             # [D, B*4]
        out[:, 4 * h:4 * h + 4, :] = o.reshape(D, B, 4).transpose(1, 2, 0)
    return out


def kernel(query, key_cache, value_cache, block_tables, context_lens):
    nc, in_maps, meta = prepare(query, key_cache, value_cache,
                                block_tables, context_lens)
    kres = run_bass_kernel_spmd(nc, in_maps, list(range(meta[3])))
    globals()["_last_results"] = kres
    return assemble(kres.results, meta)
